# revision 1
# baseline (speedup 1.0000x reference)
"""Fused Attention1d block (groupnorm -> qkv conv1x1 -> attention -> groupnorm
-> proj conv1x1 -> residual) for Trainium2, data-parallel over batch: 8 batch
elements -> 8 NeuronCores, no collectives.

Per-core layout strategy (x_b is [C=512, T=2048], channels on partitions):
  - gn stats: per-channel bn_stats (fp32), group-combine via tiny fp32 PE
    matmuls with a 0/1 selector, expand back with the transposed selector.
  - qkv matmul computes only Q,K in a packed head-pair layout (pair p tile has
    head 2p on partitions 0:64, head 2p+1 on 64:128).  V is produced directly
    TRANSPOSED (V^T [t, ch]) by a separate matmul, with a ones-column appended
    per head (weight column of zeros + bias 1.0).
  - scores are computed directly in [s, t] orientation (lhsT=k, rhs=q), so the
    whole attention needs no transposes: P = exp(S/8) unnormalized (S is
    O(+-6), max-subtract is unnecessary), and the softmax denominators fall
    out of the AV matmul for free via the ones-column (row 64 of the AV
    accumulator).  Normalization happens after AV as a per-element multiply
    with 1/l, broadcast across partitions by strided DMA.
  - all large matmuls run with bf16 operands (full PE rate, fp32 accumulate);
    statistics, softmax, normalization and the residual stay fp32.
"""

import numpy as np
import ml_dtypes

import concourse.bass as bass
import concourse.tile as tile
from concourse import bacc, mybir
from concourse.bass_utils import run_bass_kernel_spmd

AF = mybir.ActivationFunctionType
ALU = mybir.AluOpType
F32 = mybir.dt.float32
BF16 = mybir.dt.bfloat16

NCORES = 8
B, C, T = 8, 512, 2048
H = 8            # attention heads
CH = 64          # channels per head
G = 32           # groupnorm groups
GS = C // G      # 16 channels per group
EPS = 1e-5
KC = C // 128    # 4 channel chunks
TC5 = T // 512   # 4 t-chunks of 512
SC = T // 128    # 16 s-chunks of 128

# q and k are each scaled by 1/sqrt(sqrt(CH)) in the reference; we apply the
# squared scale once, inside the exp activation's free affine.
_s = np.float32(1.0) / np.sqrt(np.sqrt(np.float32(CH)))
SCALE2 = float(np.float32(_s) * np.float32(_s))

_CACHE = {}


def _bcast_rows(src_row, nrows):
    """AP that reads one [1, N] sbuf row nrows times (partition broadcast)."""
    return bass.AP(tensor=src_row.tensor, offset=src_row.offset,
                   ap=[[0, nrows], list(src_row.ap[-1])])


def _groupnorm(nc, stats, gnps, src, dst, sel_sb, selt_sb, gb_sb, gcol, eps32):
    """Group norm over 4 channel-chunk tiles. src: 4 fp32 [128,>=T] APs,
    dst: 4 [128,>=T] APs (any dtype). gb_sb [128,16]: gamma cols gcol..gcol+3,
    beta cols gcol+4..gcol+7."""
    rs_list = []
    for k in range(KC):
        st = stats.tile([128, 4, 6], F32, tag="bnst")
        for sub in range(4):
            nc.vector.bn_stats(out=st[:, sub, :],
                               in_=src[k][:, 512 * sub:512 * (sub + 1)])
        mv = stats.tile([128, 2], F32, tag="bnmv")
        nc.vector.bn_aggr(out=mv, in_=st)
        # rs = [mean, E[x^2]] per channel
        rs = stats.tile([128, 2], F32, tag="bnrs")
        nc.vector.tensor_mul(out=rs[:, 1:2], in0=mv[:, 0:1], in1=mv[:, 0:1])
        nc.vector.tensor_add(out=rs[:, 1:2], in0=rs[:, 1:2], in1=mv[:, 1:2])
        nc.vector.tensor_copy(out=rs[:, 0:1], in_=mv[:, 0:1])
        rs_list.append(rs)

    # group stats [G,2] = sum_k sel_k.T @ rs_k -> (mean_g, E2_g)
    gp = gnps.tile([G, 2], F32, tag="gps")
    for k in range(KC):
        nc.tensor.matmul(gp, lhsT=sel_sb[:, k, :], rhs=rs_list[k],
                         start=(k == 0), stop=(k == KC - 1))
    # gg rows 0:G = [mean_g, rstd_g]; rows G:128 zero
    gg = stats.tile([128, 2], F32, tag="gng")
    # partition regions starting at 32 may span at most 32 partitions
    nc.vector.memset(gg[32:64, :], 0.0)
    nc.vector.memset(gg[64:128, :], 0.0)
    nc.vector.tensor_copy(out=gg[:G, 0:1], in_=gp[:, 0:1])
    tmp = stats.tile([G, 1], F32, tag="gnt")
    # square the mean from its SBUF copy (two PSUM operands in one DVE op are
    # rejected by the BIR verifier)
    nc.vector.tensor_mul(out=tmp, in0=gg[:G, 0:1], in1=gg[:G, 0:1])
    nc.vector.tensor_tensor(out=gg[:G, 1:2], in0=gp[:, 1:2], in1=tmp,
                            op=ALU.subtract)
    # rsqrt(v+eps) = exp(-0.5*ln(v+eps)): keeps every activation in the
    # kernel inside the single natural_log_exp ACT table set (no ~2.7us
    # table reloads between gn / attention / normalize phases).
    nc.scalar.activation(out=gg[:G, 1:2], in_=gg[:G, 1:2], func=AF.Ln,
                         bias=eps32, scale=1.0)
    nc.scalar.activation(out=gg[:G, 1:2], in_=gg[:G, 1:2], func=AF.Exp,
                         scale=-0.5)

    for k in range(KC):
        ex = gnps.tile([128, 2], F32, tag="gex")
        nc.tensor.matmul(ex, lhsT=selt_sb[:, 128 * k:128 * (k + 1)], rhs=gg,
                         start=True, stop=True)
        # A = rstd*gamma ; Bc = beta - mean*A ; out = x*A + Bc
        ab = stats.tile([128, 2], F32, tag="gnab")
        nc.vector.tensor_mul(out=ab[:, 0:1], in0=ex[:, 1:2],
                             in1=gb_sb[:, gcol + k:gcol + k + 1])
        nc.vector.tensor_mul(out=ab[:, 1:2], in0=ex[:, 0:1], in1=ab[:, 0:1])
        nc.vector.tensor_tensor(out=ab[:, 1:2],
                                in0=gb_sb[:, gcol + 4 + k:gcol + 5 + k],
                                in1=ab[:, 1:2], op=ALU.subtract)
        nc.vector.tensor_scalar(out=dst[k][:, 0:T], in0=src[k][:, 0:T],
                                scalar1=ab[:, 0:1], scalar2=ab[:, 1:2],
                                op0=ALU.mult, op1=ALU.add)


def _kernel_body(nc, tc, d, out_d, reps=1):
    if reps > 1:
        with tc.For_i(0, reps, 1):
            _kernel_body_inner(nc, tc, d, out_d)
    else:
        _kernel_body_inner(nc, tc, d, out_d)


STOP_AFTER = "F"


def _kernel_body_inner(nc, tc, d, out_d):
    import contextlib
    ctx = contextlib.ExitStack()
    with ctx:
        # ---- persistent SBUF pools ----
        big4 = ctx.enter_context(tc.tile_pool(name="big4", bufs=4))
        act4 = ctx.enter_context(tc.tile_pool(name="act4", bufs=7))
        qkp = ctx.enter_context(tc.tile_pool(name="qkp", bufs=8))
        wqkp = ctx.enter_context(tc.tile_pool(name="wqkp", bufs=4))
        wvp = ctx.enter_context(tc.tile_pool(name="wvp", bufs=4))
        small = ctx.enter_context(tc.tile_pool(name="small", bufs=1))
        stats = ctx.enter_context(tc.tile_pool(name="stats", bufs=4))
        lrp = ctx.enter_context(tc.tile_pool(name="lrp", bufs=1))
        shif = ctx.enter_context(tc.tile_pool(name="shif", bufs=2))
        rbp = ctx.enter_context(tc.tile_pool(name="rbp", bufs=3))
        outp = ctx.enter_context(tc.tile_pool(name="outp", bufs=4))

        # ---- constants / weights ----
        sel_sb = small.tile([128, KC, G], F32)
        nc.sync.dma_start(sel_sb, d["sel"].rearrange("(k p) g -> p k g", p=128))
        selt_sb = small.tile([128, C], F32)
        nc.sync.dma_start(selt_sb, d["selt"])
        gb_sb = small.tile([128, 16], F32)  # g1[0:4] b1[4:8] g2[8:12] b2[12:16]
        nc.sync.dma_start(gb_sb[:, 0:4], d["g1"].rearrange("(k p) -> p k", p=128))
        nc.sync.dma_start(gb_sb[:, 4:8], d["b1"].rearrange("(k p) -> p k", p=128))
        nc.sync.dma_start(gb_sb[:, 8:12], d["g2"].rearrange("(k p) -> p k", p=128))
        nc.sync.dma_start(gb_sb[:, 12:16], d["b2"].rearrange("(k p) -> p k", p=128))
        bqk_sb = small.tile([128, H], F32)
        nc.sync.dma_start(bqk_sb, d["bqk"].rearrange("(m p) -> p m", p=128))
        bp_sb = small.tile([128, KC], F32)
        nc.sync.dma_start(bp_sb, d["bp"].rearrange("(m p) -> p m", p=128))
        bv_sb = small.tile([128, 520], F32)
        bv_bcast = bass.AP(tensor=d["bv"].tensor, offset=d["bv"].offset,
                           ap=[[0, 128]] + [list(a) for a in d["bv"].ap])
        nc.sync.dma_start(bv_sb, bv_bcast)
        eps32 = small.tile([G, 1], F32)
        nc.vector.memset(eps32, EPS)

        wqk_sb = [wqkp.tile([128, 1024], BF16, tag="wqkpt", name=f"wqk{i}")
                  for i in range(KC)]
        for k in range(KC):
            nc.sync.dma_start(wqk_sb[k], d["wqk"][128 * k:128 * (k + 1), :])
        wv_sb = [wvp.tile([128, 520], BF16, tag="wvwp", name=f"wv{i}")
                 for i in range(KC)]
        for k in range(KC):
            nc.sync.dma_start(wv_sb[k], d["wv"][128 * k:128 * (k + 1), :])

        # ---- load x ----
        xt = [big4.tile([128, 2080], F32, tag="big", name=f"xt{i}")
              for i in range(KC)]
        for k in range(KC):
            nc.sync.dma_start(xt[k][:, 0:T], d["x"][128 * k:128 * (k + 1), :])

        xn = [act4.tile([128, T], BF16, tag="act", name=f"xn{i}")
              for i in range(KC)]

        # ---- phase A: gn1 (x -> xn, bf16) ----
        with tc.tile_pool(name="gnps1", bufs=2, space="PSUM") as gnps:
            _groupnorm(nc, stats, gnps, xt, xn, sel_sb, selt_sb, gb_sb, 0, eps32)

        if STOP_AFTER == "A":
            return
        # ---- phase B: packed Q/K matmul + V^T matmul ----
        qk_sb = [qkp.tile([128, T], BF16, tag="qk", name=f"qk{i}")
                 for i in range(H)]
        vt_sb = [big4.tile([128, 2080], BF16, tag="big", name=f"vt{i}")
                 for i in range(4)]
        def qk_tile(mt, pool, tag):
            for n in range(TC5):
                ps = pool.tile([128, 512], F32, tag=tag, bufs=2, name="psqk")
                for k in range(KC):
                    nc.tensor.matmul(
                        ps, lhsT=wqk_sb[k][:, 128 * mt:128 * (mt + 1)],
                        rhs=xn[k][:, 512 * n:512 * (n + 1)],
                        start=(k == 0), stop=(k == KC - 1))
                nc.vector.tensor_scalar(
                    out=qk_sb[mt][:, 512 * n:512 * (n + 1)], in0=ps,
                    scalar1=bqk_sb[:, mt:mt + 1], scalar2=None,
                    op0=ALU.add, op1=ALU.bypass)

        with tc.tile_pool(name="mmps", bufs=3, space="PSUM") as mmps:
            def vt_chunk(bsc):
                # V^T: t-chunk b -> vt_sb[b//4][:, 520*(b%4) : 520*(b%4)+520]
                ps = mmps.tile([128, 512], F32, tag="mmvt", bufs=2, name="psvt")
                pr = mmps.tile([128, 8], F32, tag="mmvr", bufs=2, name="psvr")
                for k in range(KC):
                    lhsT = xn[k][:, 128 * bsc:128 * (bsc + 1)]
                    nc.tensor.matmul(ps, lhsT=lhsT, rhs=wv_sb[k][:, 0:512],
                                     start=(k == 0), stop=(k == KC - 1))
                    nc.tensor.matmul(pr, lhsT=lhsT, rhs=wv_sb[k][:, 512:520],
                                     start=(k == 0), stop=(k == KC - 1))
                c0 = 520 * (bsc % 4)
                nc.vector.tensor_add(out=vt_sb[bsc // 4][:, c0:c0 + 512],
                                     in0=ps, in1=bv_sb[:, 0:512])
                nc.vector.tensor_add(out=vt_sb[bsc // 4][:, c0 + 512:c0 + 520],
                                     in0=pr, in1=bv_sb[:, 512:520])

            # heads 0/1's Q,K and all of V^T first so attention starts early;
            # remaining Q/K tiles are interleaved into the attention stream.
            qk_tile(0, mmps, "mmqk")
            qk_tile(1, mmps, "mmqk")
            for bsc in range(SC):
                vt_chunk(bsc)

        if STOP_AFTER == "B":
            return
        # ---- phase C: attention ----
        h_sb = [act4.tile([128, T], F32, tag="act", name=f"hsb{i}")
                for i in range(H // 2)]
        l_sb = lrp.tile([128, T], F32, tag="lsb")
        pending_qk = list(range(2, H))
        with tc.tile_pool(name="scps", bufs=4, space="PSUM") as scps, \
             tc.tile_pool(name="avps", bufs=2, space="PSUM") as avps:
            for h in range(H):
                p2 = h // 2
                r0 = 64 * (h % 2)
                qt = qk_sb[2 * p2]
                kt = qk_sb[2 * p2 + 1]
                ltmp = shif.tile([65, T], F32, tag="ltmp", name=f"lt{h}")
                for n2 in range(2):
                    t0 = 1024 * n2
                    av = avps.tile([128, 1024], F32, tag="av", bufs=2)
                    for bsc in range(SC):
                        sc_t = scps.tile([128, 1024], F32, tag="sc", bufs=2)
                        for j in range(2):
                            nc.tensor.matmul(
                                sc_t[:, 512 * j:512 * (j + 1)],
                                lhsT=kt[r0:r0 + 64, 128 * bsc:128 * (bsc + 1)],
                                rhs=qt[r0:r0 + 64,
                                       t0 + 512 * j:t0 + 512 * (j + 1)],
                                start=True, stop=True)
                        pt = wqkp.tile([128, 1024], BF16, tag="pt", bufs=6)
                        nc.scalar.activation(out=pt, in_=sc_t, func=AF.Exp,
                                             scale=SCALE2)
                        vslice = vt_sb[bsc // 4][:, 520 * (bsc % 4) + 65 * h:
                                                 520 * (bsc % 4) + 65 * (h + 1)]
                        for j in range(2):
                            nc.tensor.matmul(
                                av[0:65, 512 * j:512 * (j + 1)], lhsT=vslice,
                                rhs=pt[:, 512 * j:512 * (j + 1)],
                                start=(bsc == 0), stop=(bsc == SC - 1))
                    # evict A rows (32-aligned partition shift is legal on
                    # DVE) and stash the l row (row 64 -> row h needs DMA).
                    nc.vector.tensor_copy(
                        out=h_sb[p2][r0:r0 + 64, t0:t0 + 1024], in_=av[0:64, :])
                    nc.vector.tensor_copy(
                        out=ltmp[64:65, t0:t0 + 1024], in_=av[64:65, :])
                    # feed one deferred Q/K tile into the stream, borrowing
                    # scores psum slots (keeps total PSUM at 8 banks)
                    if pending_qk:
                        qk_tile(pending_qk.pop(0), avps, "av")
                nc.sync.dma_start(l_sb[h:h + 1, :], ltmp[64:65, :])

        if STOP_AFTER == "C":
            return
        # ---- phase D: normalize by 1/l ----
        r_sb = lrp.tile([8, T], F32, tag="rsb")
        # 1/l as exp(-ln(l)): l is a sum of positives in [~5e2, ~5e3]; both
        # functions live in one ACT table set.
        nc.scalar.activation(out=r_sb, in_=l_sb[:H, :], func=AF.Ln)
        nc.scalar.activation(out=r_sb, in_=r_sb, func=AF.Exp, scale=-1.0)
        # partition-broadcast needs a DRAM source (SBUF APs require nonzero
        # partition step), so bounce r through DRAM.
        with tc.tile_pool(name="rdram", bufs=1, space="DRAM") as drp:
            rd = drp.tile([8, T], F32)
            nc.sync.dma_start(rd, r_sb)
            for p2 in range(H // 2):
                rbb = rbp.tile([128, T], F32, tag="rbb", bufs=2)
                nc.sync.dma_start(rbb[0:64, :],
                                  _bcast_rows(rd[2 * p2:2 * p2 + 1, :], 64))
                nc.sync.dma_start(rbb[64:128, :],
                                  _bcast_rows(rd[2 * p2 + 1:2 * p2 + 2, :], 64))
                for n in range(TC5):
                    nc.vector.tensor_mul(
                        out=h_sb[p2][:, 512 * n:512 * (n + 1)],
                        in0=h_sb[p2][:, 512 * n:512 * (n + 1)],
                        in1=rbb[:, 512 * n:512 * (n + 1)])

        if STOP_AFTER == "D":
            return
        # ---- phase E: gn2 (h_sb fp32 -> hn bf16) ----
        hn_sb = [qkp.tile([128, T], BF16, tag="qk", name=f"hn{i}")
                 for i in range(KC)]
        with tc.tile_pool(name="gnps2", bufs=2, space="PSUM") as gnps:
            _groupnorm(nc, stats, gnps, h_sb, hn_sb, sel_sb, selt_sb, gb_sb, 8,
                       eps32)

        if STOP_AFTER == "E":
            return
        # ---- phase F: proj + bias + residual ----
        wp_sb = [wvp.tile([128, 520], BF16, tag="wvwp", name=f"wp{i}")
                 for i in range(KC)]
        for k in range(KC):
            nc.sync.dma_start(wp_sb[k][:, 0:512], d["wp"][128 * k:128 * (k + 1), :])
        xres = [big4.tile([128, 2080], F32, tag="big", name=f"xres{i}")
                for i in range(KC)]
        for k in range(KC):
            nc.sync.dma_start(xres[k][:, 0:T], d["x"][128 * k:128 * (k + 1), :])
        with tc.tile_pool(name="prps", bufs=3, space="PSUM") as prps:
            for mt in range(KC):
                for n in range(TC5):
                    ps = prps.tile([128, 512], F32, tag="pr")
                    for k in range(KC):
                        nc.tensor.matmul(
                            ps, lhsT=wp_sb[k][:, 128 * mt:128 * (mt + 1)],
                            rhs=hn_sb[k][:, 512 * n:512 * (n + 1)],
                            start=(k == 0), stop=(k == KC - 1))
                    ot = outp.tile([128, 512], F32, tag="ot")
                    nc.vector.scalar_tensor_tensor(
                        out=ot, in0=ps, scalar=bp_sb[:, mt:mt + 1],
                        in1=xres[mt][:, 512 * n:512 * (n + 1)],
                        op0=ALU.add, op1=ALU.add)
                    nc.sync.dma_start(
                        out_d[128 * mt:128 * (mt + 1), 512 * n:512 * (n + 1)], ot)


def _build_module(reps=1):
    nc = bacc.Bacc("TRN2", target_bir_lowering=False, debug=False,
                   num_devices=NCORES)
    d = {}

    def inp(name, shape, dt=F32):
        d[name] = nc.dram_tensor(name, shape, dt, kind="ExternalInput").ap()

    inp("x", [C, T])
    inp("wqk", [C, 1024], BF16)
    inp("bqk", [1024])
    inp("wv", [C, 520], BF16)
    inp("bv", [520])
    inp("wp", [C, C], BF16)
    inp("bp", [C])
    inp("g1", [C]); inp("b1", [C]); inp("g2", [C]); inp("b2", [C])
    inp("sel", [C, G])
    inp("selt", [128, C])
    out_d = nc.dram_tensor("out", [C, T], F32, kind="ExternalOutput").ap()

    with tile.TileContext(nc) as tc:
        _kernel_body(nc, tc, d, out_d, reps=reps)
    nc.compile()
    return nc


def _prep_weights(w_qkv, b_qkv, w_proj, b_proj):
    w_qkv = np.asarray(w_qkv, np.float32)
    b_qkv = np.asarray(b_qkv, np.float32)
    q = [w_qkv[192 * h:192 * h + 64] for h in range(H)]
    k = [w_qkv[192 * h + 64:192 * h + 128] for h in range(H)]
    v = [w_qkv[192 * h + 128:192 * h + 192] for h in range(H)]
    qb = [b_qkv[192 * h:192 * h + 64] for h in range(H)]
    kb = [b_qkv[192 * h + 64:192 * h + 128] for h in range(H)]
    vb = [b_qkv[192 * h + 128:192 * h + 192] for h in range(H)]

    wqk_rows = []
    bqk = []
    for p in range(H // 2):
        wqk_rows += [q[2 * p], q[2 * p + 1], k[2 * p], k[2 * p + 1]]
        bqk += [qb[2 * p], qb[2 * p + 1], kb[2 * p], kb[2 * p + 1]]
    wqk = np.ascontiguousarray(np.concatenate(wqk_rows, 0).T)      # [512,1024]
    bqk = np.concatenate(bqk, 0)                                   # [1024]

    wv = np.zeros((C, 520), np.float32)
    bv = np.zeros((520,), np.float32)
    for h in range(H):
        wv[:, 65 * h:65 * h + 64] = v[h].T
        bv[65 * h:65 * h + 64] = vb[h]
        bv[65 * h + 64] = 1.0

    wp = np.ascontiguousarray(np.asarray(w_proj, np.float32).T)
    bp = np.asarray(b_proj, np.float32)

    sel = np.zeros((C, G), np.float32)
    sel[np.arange(C), np.arange(C) // GS] = 1.0 / GS
    selt = np.zeros((128, C), np.float32)
    selt[np.arange(C) // GS, np.arange(C)] = 1.0

    bf = ml_dtypes.bfloat16
    return dict(wqk=wqk.astype(bf), bqk=bqk, wv=wv.astype(bf), bv=bv,
                wp=wp.astype(bf), bp=bp, sel=sel, selt=selt)


def _make_in_maps(x, gn1_gamma, gn1_beta, w_qkv, b_qkv, gn2_gamma, gn2_beta,
                  w_proj, b_proj):
    x = np.asarray(x, np.float32)
    shared = _prep_weights(w_qkv, b_qkv, w_proj, b_proj)
    shared.update(g1=np.asarray(gn1_gamma, np.float32),
                  b1=np.asarray(gn1_beta, np.float32),
                  g2=np.asarray(gn2_gamma, np.float32),
                  b2=np.asarray(gn2_beta, np.float32))
    return [dict(shared, x=np.ascontiguousarray(x[c])) for c in range(NCORES)]


def kernel(x, gn1_gamma, gn1_beta, w_qkv, b_qkv, gn2_gamma, gn2_beta, w_proj,
           b_proj):
    if "nc" not in _CACHE:
        _CACHE["nc"] = _build_module(reps=1)
    nc = _CACHE["nc"]
    in_maps = _make_in_maps(x, gn1_gamma, gn1_beta, w_qkv, b_qkv, gn2_gamma,
                            gn2_beta, w_proj, b_proj)
    res = run_bass_kernel_spmd(nc, in_maps, core_ids=list(range(NCORES)))
    out = np.stack([res.results[c]["out"] for c in range(NCORES)], 0)
    return out.astype(np.float32)


def _make_runner(nc, in_maps):
    """Cached jitted executor with device-resident inputs; per-call cost is
    dispatch + device execution only (no host transfers, no retrace)."""
    import jax
    import jax.numpy as jnp
    from jax.experimental.shard_map import shard_map
    from jax.sharding import Mesh, PartitionSpec, NamedSharding
    from concourse import bass2jax, mybir as mb

    bass2jax.install_neuronx_cc_hook()
    part_name = nc.partition_id_tensor.name if nc.partition_id_tensor else None
    in_names, out_names, out_avals, zero_outs = [], [], [], []
    for alloc in nc.m.functions[0].allocations:
        if not isinstance(alloc, mb.MemoryLocationSet):
            continue
        name = alloc.memorylocations[0].name
        if alloc.kind == "ExternalInput":
            if name != part_name:
                in_names.append(name)
        elif alloc.kind == "ExternalOutput":
            out_names.append(name)
            shape = tuple(alloc.tensor_shape)
            dtype = mb.dt.np(alloc.dtype)
            out_avals.append(jax.core.ShapedArray(shape, dtype))
            zero_outs.append(np.zeros(shape, dtype))
    n_params = len(in_names)
    all_names = in_names + out_names + ([part_name] if part_name else [])

    def _body(*args):
        operands = list(args)
        if part_name:
            operands.append(bass2jax.partition_id_tensor())
        outs = bass2jax._bass_exec_p.bind(
            *operands, out_avals=tuple(out_avals), in_names=tuple(all_names),
            out_names=tuple(out_names), lowering_input_output_aliases=(),
            sim_require_finite=True, sim_require_nnan=True, nc=nc)
        return tuple(outs)

    devices = jax.devices()[:NCORES]
    mesh = Mesh(np.asarray(devices), ("core",))
    spec = PartitionSpec("core")
    fn = jax.jit(shard_map(_body, mesh=mesh,
                           in_specs=(spec,) * (n_params + len(out_names)),
                           out_specs=(spec,) * len(out_names),
                           check_rep=False), keep_unused=True)
    sh = NamedSharding(mesh, spec)
    dev_args = [
        jax.device_put(
            np.concatenate([np.asarray(in_maps[c][nm])[None] for c in
                            range(NCORES)], 0).reshape(
                NCORES * np.asarray(in_maps[0][nm]).shape[0],
                *np.asarray(in_maps[0][nm]).shape[1:]), sh)
        for nm in in_names
    ] + [
        jax.device_put(np.zeros((NCORES * z.shape[0], *z.shape[1:]), z.dtype),
                       sh) for z in zero_outs
    ]

    def call():
        outs = fn(*dev_args)
        jax.block_until_ready(outs)
        return outs

    return call


def bench(inputs, rep_list=(1, 33), n_calls=5):
    """Estimate on-device kernel time by the slope method: per-call wall time
    of an R-rep hardware loop for two R values; the difference cancels
    dispatch overhead."""
    import time
    in_maps = _make_in_maps(**inputs)
    walls = {}
    for reps in rep_list:
        key = f"nc{reps}"
        if key not in _CACHE:
            _CACHE[key] = _build_module(reps=reps)
        runner = _make_runner(_CACHE[key], in_maps)
        runner()  # warmup (compile+load)
        times = []
        for _ in range(n_calls):
            t0 = time.time()
            runner()
            times.append(time.time() - t0)
        walls[reps] = min(times)
        print(f"reps={reps}: call walls {[f'{t*1e3:.1f}ms' for t in times]}",
              flush=True)
    lo, hi = min(rep_list), max(rep_list)
    est = (walls[hi] - walls[lo]) / (hi - lo)
    print(f"estimated per-iteration kernel time: {est * 1e9:.0f} ns")
    return est



# revision 28
# speedup vs baseline: 1.1465x; 1.1465x over previous
"""Fused Attention1d block (groupnorm -> qkv conv1x1 -> attention -> groupnorm
-> proj conv1x1 -> residual) for Trainium2, data-parallel over batch: 8 batch
elements -> 8 NeuronCores, no collectives.

v2 design (per-core, x_b is [C=512, T=2048], channels on partitions):
  - attention processed in head PAIRS: the two heads' score matmuls use
    contraction rows 0:64 / 64:128, so the PE runs them CONCURRENTLY via
    row tiling (tile_position auto-derived from base partitions).
  - softmax exp is split across BOTH lane engines: ACT does true exp;
    DVE computes a Schraudolph-style bit-trick exp (int16(S*K1+K2)
    reinterpreted as bf16) via tensor_scalar + bitcast.  The split ratio
    alternates per s-chunk; errors are ~+-3% zero-ish-mean and cancel in
    the A/l softmax ratio (measured end-to-end ~7e-3 rel).
  - AV accumulates [A(64 ch); l(ones-row)] per (head, 512-t window); one
    [65,512] eviction drops A+l into a per-head fp32 scratch; l rows are
    gathered to a [8,T] tile by tiny SBUF->SBUF DMAs (no engine time).
  - r = 1/l via one ACT Ln + Exp(-1) per pair; r is partition-broadcast
    by a tiny PE matmul with a 0/1 selector (no DRAM bounce); the
    normalize multiply fuses the broadcast PSUM operand.
  - gn2 is computed CHUNK-LOCALLY per head pair (groups never straddle
    heads), so normalize+gn2 for pair p overlap pair p+1's attention.
  - emission is software-pipelined: S(u) || exp(u) || AV(u-1), with VT /
    deferred-QK / normalize / gn2 work injected into the attention stream
    as fillers so PE never starves while ACT+DVE chew on exp.
"""

import numpy as np
import ml_dtypes

import concourse.bass as bass
import concourse.tile as tile
from concourse import bacc, mybir
from concourse.bass_utils import run_bass_kernel_spmd

AF = mybir.ActivationFunctionType
ALU = mybir.AluOpType
F32 = mybir.dt.float32
BF16 = mybir.dt.bfloat16
I16 = mybir.dt.int16

NCORES = 8
B, C, T = 8, 512, 2048
H = 8            # attention heads
P2 = H // 2      # head pairs
CH = 64          # channels per head
G = 32           # groupnorm groups
GS = C // G      # 16 channels per group
EPS = 1e-5
KC = C // 128    # 4 channel chunks
TW = 4           # t-windows of 512
SC = T // 128    # 16 s-chunks of 128

# q and k are each scaled by 1/sqrt(sqrt(CH)); squared scale applied once in
# the exp input.
SCALE2 = 0.125
LOG2E = float(np.log2(np.e))
EXP_K1 = SCALE2 * 128.0 * LOG2E          # Schraudolph multiplier (bf16 bits)
EXP_K2 = 127.0 * 128.0 - 5.0             # exponent bias + centering offset
# which of every 8 attention units run the DVE bit-trick exp (rest on ACT)
DVE_SET = (1, 3, 5, 7)

_CACHE = {}


def _dedupe_act_loads(nc, set_id=6):
    """Point every ACT table load at natural_log_exp_and_others (covers both
    Ln and Exp) and drop all but the first load per block: the compiler pass
    assigns Exp->set 0 and Ln->set 5, thrashing ~2.7us per switch."""
    for f in nc.m.functions:
        for block in f.blocks:
            first = True
            keep = []
            for inst in block.instructions:
                if inst.__class__.__name__ == "InstLoadActFuncSet":
                    assert inst.sync_info is None
                    if not first:
                        continue
                    inst.act_func_set_id = set_id
                    first = False
                keep.append(inst)
            block.instructions[:] = keep


def _kernel_body(nc, tc, d, out_d, reps=1):
    if reps > 1:
        with tc.For_i(0, reps, 1):
            _kernel_body_inner(nc, tc, d, out_d)
    else:
        _kernel_body_inner(nc, tc, d, out_d)


def _kernel_body_inner(nc, tc, d, out_d):
    import contextlib
    ctx = contextlib.ExitStack()
    with ctx:
        # ---- persistent SBUF pools ----
        small = ctx.enter_context(tc.tile_pool(name="small", bufs=1))
        wqkp = ctx.enter_context(tc.tile_pool(name="wqkp", bufs=4))
        wvp = ctx.enter_context(tc.tile_pool(name="wvp", bufs=4))
        wpp = ctx.enter_context(tc.tile_pool(name="wpp", bufs=4))
        xhp = ctx.enter_context(tc.tile_pool(name="xhp", bufs=4))
        xnp = ctx.enter_context(tc.tile_pool(name="xnp", bufs=4))
        qkp = ctx.enter_context(tc.tile_pool(name="qkp", bufs=4))
        vtp = ctx.enter_context(tc.tile_pool(name="vtp", bufs=16))
        ptp = ctx.enter_context(tc.tile_pool(name="ptp", bufs=4))
        s65p = ctx.enter_context(tc.tile_pool(name="s65p", bufs=4))
        hnp = ctx.enter_context(tc.tile_pool(name="hnp", bufs=4))
        lrp = ctx.enter_context(tc.tile_pool(name="lrp", bufs=2))
        rtp = ctx.enter_context(tc.tile_pool(name="rtp", bufs=1))
        stats = ctx.enter_context(tc.tile_pool(name="stats", bufs=4))
        xrp = ctx.enter_context(tc.tile_pool(name="xrp", bufs=6))
        outp = ctx.enter_context(tc.tile_pool(name="outp", bufs=4))
        # PSUM: sc 2x[128,1024]=4 banks, av 2x2x[65,512]=4 banks
        scps = ctx.enter_context(tc.tile_pool(name="scps", bufs=2, space="PSUM"))
        avps = ctx.enter_context(tc.tile_pool(name="avps", bufs=2, space="PSUM"))

        # ---- constants / weights ----
        selg_sb = small.tile([128, 8], F32)      # chunk-local group combine
        nc.sync.dma_start(selg_sb, d["selg"])
        selgt_sb = small.tile([8, 128], F32)     # chunk-local group expand
        nc.sync.dma_start(selgt_sb, d["selgt"])
        sel2_sb = small.tile([8, 128], BF16)     # r partition-broadcast
        nc.sync.dma_start(sel2_sb, d["sel2"])
        gb_sb = small.tile([128, 16], F32)  # g1[0:4] b1[4:8] g2[8:12] b2[12:16]
        nc.sync.dma_start(gb_sb[:, 0:4], d["g1"].rearrange("(k p) -> p k", p=128))
        nc.sync.dma_start(gb_sb[:, 4:8], d["b1"].rearrange("(k p) -> p k", p=128))
        nc.sync.dma_start(gb_sb[:, 8:12], d["g2"].rearrange("(k p) -> p k", p=128))
        nc.sync.dma_start(gb_sb[:, 12:16], d["b2"].rearrange("(k p) -> p k", p=128))
        bqk_sb = small.tile([128, H], F32)
        nc.sync.dma_start(bqk_sb, d["bqk"].rearrange("(m p) -> p m", p=128))
        bp_sb = small.tile([128, KC], F32)
        nc.sync.dma_start(bp_sb, d["bp"].rearrange("(m p) -> p m", p=128))
        bv_sb = small.tile([128, 520], F32)
        bv_bcast = bass.AP(tensor=d["bv"].tensor, offset=d["bv"].offset,
                           ap=[[0, 128]] + [list(a) for a in d["bv"].ap])
        nc.sync.dma_start(bv_sb, bv_bcast)
        eps8 = small.tile([8, 1], F32)
        nc.vector.memset(eps8, EPS)

        wqk_sb = [wqkp.tile([128, 1024], BF16, tag="wqk", name=f"wqk{i}")
                  for i in range(KC)]
        for k in range(KC):
            nc.sync.dma_start(wqk_sb[k], d["wqk"][128 * k:128 * (k + 1), :])
        wv_sb = [wvp.tile([128, 520], BF16, tag="wv", name=f"wv{i}")
                 for i in range(KC)]
        for k in range(KC):
            nc.sync.dma_start(wv_sb[k], d["wv"][128 * k:128 * (k + 1), :])
        wp_sb = [wpp.tile([128, 512], BF16, tag="wp", name=f"wp{i}")
                 for i in range(KC)]
        for k in range(KC):
            nc.sync.dma_start(wp_sb[k], d["wp"][128 * k:128 * (k + 1), :])

        # ---- load x, chunk-local gn1 -> xn (bf16) ----
        xt = [xhp.tile([128, T], F32, tag="xh", name=f"xt{i}")
              for i in range(KC)]
        for k in range(KC):
            nc.sync.dma_start(xt[k], d["x"][128 * k:128 * (k + 1), :])
        xn = [xnp.tile([128, T], BF16, tag="xn", name=f"xn{i}")
              for i in range(KC)]

        def gn_finish(gp_rhs, gcol_g, gcol_b, src, dst):
            """Shared per-chunk groupnorm combine+apply: per-channel (mean,E2)
            in gp_rhs -> group rstd -> affine apply src->dst."""
            gp = avps.tile([8, 2], F32, tag="av_e", name="gnp")
            nc.tensor.matmul(gp, lhsT=selg_sb, rhs=gp_rhs, start=True,
                             stop=True)
            gg = stats.tile([8, 2], F32, tag="gng")
            nc.vector.tensor_copy(out=gg[:, 0:1], in_=gp[:, 0:1])
            tmp = stats.tile([8, 1], F32, tag="gnt")
            nc.vector.tensor_mul(out=tmp, in0=gg[:, 0:1], in1=gg[:, 0:1])
            nc.vector.tensor_tensor(out=gg[:, 1:2], in0=gp[:, 1:2], in1=tmp,
                                    op=ALU.subtract)
            nc.scalar.activation(out=gg[:, 1:2], in_=gg[:, 1:2], func=AF.Ln,
                                 bias=eps8, scale=1.0)
            nc.scalar.activation(out=gg[:, 1:2], in_=gg[:, 1:2], func=AF.Exp,
                                 scale=-0.5)
            ex = avps.tile([128, 2], F32, tag="av_o", name="gne")
            nc.tensor.matmul(ex, lhsT=selgt_sb, rhs=gg, start=True, stop=True)
            ab = stats.tile([128, 2], F32, tag="gnab")
            nc.vector.tensor_mul(out=ab[:, 0:1], in0=ex[:, 1:2],
                                 in1=gb_sb[:, gcol_g:gcol_g + 1])
            nc.vector.tensor_mul(out=ab[:, 1:2], in0=ex[:, 0:1],
                                 in1=ab[:, 0:1])
            nc.vector.tensor_tensor(out=ab[:, 1:2],
                                    in0=gb_sb[:, gcol_b:gcol_b + 1],
                                    in1=ab[:, 1:2], op=ALU.subtract)
            if dst is not None:
                nc.any.tensor_scalar(out=dst, in0=src, scalar1=ab[:, 0:1],
                                     scalar2=ab[:, 1:2], op0=ALU.mult,
                                     op1=ALU.add)
            return ab

        def rs_from_stats(st):
            mv = stats.tile([128, 2], F32, tag="bnmv")
            nc.vector.bn_aggr(out=mv, in_=st)
            rs = stats.tile([128, 2], F32, tag="bnrs")
            nc.vector.tensor_mul(out=rs[:, 1:2], in0=mv[:, 0:1], in1=mv[:, 0:1])
            nc.vector.tensor_add(out=rs[:, 1:2], in0=rs[:, 1:2], in1=mv[:, 1:2])
            nc.vector.tensor_copy(out=rs[:, 0:1], in_=mv[:, 0:1])
            return rs

        for k in range(KC):
            st = stats.tile([128, 4, 6], F32, tag="bnst")
            for sub in range(4):
                nc.vector.bn_stats(out=st[:, sub, :],
                                   in_=xt[k][:, 512 * sub:512 * (sub + 1)])
            gn_finish(rs_from_stats(st), k, 4 + k, xt[k], xn[k])

        # ---- attention-global state ----
        qk_sb = {}          # tile index -> [128, T] bf16 (4-slot rotation)
        vt_sb = [vtp.tile([128, 520], BF16, tag="vt", name=f"vt{i}")
                 for i in range(SC)]
        # l/r rows live on partitions 0:2 (head parity: row 0 = even head),
        # per-pair tiles, so broadcast matmuls sit at base partition 0.
        l_t = {}            # pair -> [2, T] bf16
        r_t = {}            # pair -> [2, T] bf16
        ht = [None] * KC    # normalized attention out (bf16), chunk per pair
        hn = [hnp.tile([128, T], BF16, tag="hn", name=f"hn{i}")
              for i in range(KC)]
        s65 = {}            # head -> [65, T] bf16 scratch (A rows + l row)
        xres = {}           # (mt, tw) -> [128, 512] residual tile

        def qk_piece(mt, n):
            """One quarter of a Q/K tile: 4 matmuls + eviction."""
            if mt not in qk_sb:
                qk_sb[mt] = qkp.tile([128, T], BF16, tag="qk", name=f"qk{mt}")
            ps = scps.tile([128, 1024], F32, tag="sc", name="qkps")
            for k in range(KC):
                nc.tensor.matmul(
                    ps[:, 0:512], lhsT=wqk_sb[k][:, 128 * mt:128 * (mt + 1)],
                    rhs=xn[k][:, 512 * n:512 * (n + 1)],
                    start=(k == 0), stop=(k == KC - 1))
            nc.any.tensor_scalar(
                out=qk_sb[mt][:, 512 * n:512 * (n + 1)], in0=ps[:, 0:512],
                scalar1=bqk_sb[:, mt:mt + 1], scalar2=None,
                op0=ALU.add, op1=ALU.bypass)

        def vt_chunk(bsc):
            """V^T s-chunk: [t128, 512 v-cols + 8 ones-cols] -> vt_sb[bsc]."""
            ps = scps.tile([128, 1024], F32, tag="sc", name="vtps")
            for k in range(KC):
                lhsT = xn[k][:, 128 * bsc:128 * (bsc + 1)]
                nc.tensor.matmul(ps[:, 0:512], lhsT=lhsT, rhs=wv_sb[k][:, 0:512],
                                 start=(k == 0), stop=(k == KC - 1))
                nc.tensor.matmul(ps[:, 512:520], lhsT=lhsT,
                                 rhs=wv_sb[k][:, 512:520],
                                 start=(k == 0), stop=(k == KC - 1))
            nc.any.tensor_add(out=vt_sb[bsc], in0=ps[:, 0:520], in1=bv_sb)

        def r_slice(p2, tw):
            """r rows (bf16) for one t-window of a pair: r = exp(-ln l)."""
            ts = slice(512 * tw, 512 * (tw + 1))
            if p2 not in r_t:
                r_t[p2] = lrp.tile([2, T], BF16, tag="rsb", name=f"r{p2}")
            rtmp = rtp.tile([2, 512], F32, tag="rtmp")
            nc.scalar.activation(out=rtmp, in_=l_t[p2][:, ts], func=AF.Ln)
            nc.scalar.activation(out=r_t[p2][:, ts], in_=rtmp, func=AF.Exp,
                                 scale=-1.0)

        def norm_tw(p2, tw):
            """Normalize A by r for one t-window of a pair -> ht chunk (bf16)."""
            ts = slice(512 * tw, 512 * (tw + 1))
            rbb = scps.tile([128, 1024], F32, tag="sc", name="rbb")
            nc.tensor.matmul(rbb[:, 0:512], lhsT=sel2_sb[0:2, :],
                             rhs=r_t[p2][:, ts], start=True, stop=True)
            for i, h in enumerate((2 * p2, 2 * p2 + 1)):
                r0 = 64 * i
                nc.any.tensor_tensor(
                    out=ht[p2][r0:r0 + 64, ts], in0=s65[h][0:64, ts],
                    in1=rbb[r0:r0 + 64, 0:512], op=ALU.mult)

        gn2_st = {}
        gn2_ab = {}

        def gn2_stats(p2, sub):
            if p2 not in gn2_st:
                gn2_st[p2] = stats.tile([128, 4, 6], F32, tag="bnst2",
                                        name=f"st2_{p2}")
            nc.vector.bn_stats(out=gn2_st[p2][:, sub, :],
                               in_=ht[p2][:, 512 * sub:512 * (sub + 1)])

        def gn2_combine(p2):
            gn2_ab[p2] = gn_finish(rs_from_stats(gn2_st[p2]), 8 + p2, 12 + p2,
                                   None, None)

        def gn2_apply(p2, sub):
            ts = slice(512 * sub, 512 * (sub + 1))
            ab = gn2_ab[p2]
            nc.any.tensor_scalar(out=hn[p2][:, ts], in0=ht[p2][:, ts],
                                 scalar1=ab[:, 0:1], scalar2=ab[:, 1:2],
                                 op0=ALU.mult, op1=ALU.add)

        def xres_fetch(mt, tw):
            t = xrp.tile([128, 512], F32, tag="xres")
            nc.sync.dma_start(
                t, d["x"][128 * mt:128 * (mt + 1), 512 * tw:512 * (tw + 1)])
            xres[(mt, tw)] = t

        # ---- QK for pair 0 ----
        for mt in range(2):
            for n in range(4):
                qk_piece(mt, n)

        # ---- attention main loop ----
        # unit u of pair p2: tw = u // 16, bsc = u % 16
        pend_av = None   # (p2, tw, bsc, pt tile, (av_e, av_o))

        def emit_av(p2, tw, bsc, pt, tiles):
            for i, h in enumerate((2 * p2, 2 * p2 + 1)):
                nc.tensor.matmul(
                    tiles[i][0:65, :], lhsT=vt_sb[bsc][:, 65 * h:65 * (h + 1)],
                    rhs=pt[:, 512 * i:512 * (i + 1)],
                    start=(bsc == 0), stop=(bsc == SC - 1))

        def flush_av(pend):
            """Emit the pending unit's AV; on the last s-chunk also evict."""
            p2, tw, bsc, pt, tiles = pend
            emit_av(p2, tw, bsc, pt, tiles)
            if bsc == SC - 1:
                ts = slice(512 * tw, 512 * (tw + 1))
                if p2 not in l_t:
                    l_t[p2] = lrp.tile([2, T], BF16, tag="lsb", name=f"l{p2}")
                for i, h in enumerate((2 * p2, 2 * p2 + 1)):
                    if h not in s65:
                        s65[h] = s65p.tile([65, T], BF16, tag="s65",
                                           name=f"s65_{h}")
                    nc.any.tensor_copy(out=s65[h][:, ts], in_=tiles[i])
                    nc.sync.dma_start(l_t[p2][i:i + 1, ts],
                                      s65[h][64:65, ts])

        for p2 in range(P2):
            qt, kt = qk_sb[2 * p2], qk_sb[2 * p2 + 1]
            # filler schedule for this pair: unit -> [closures]
            fill = {}

            def add_fill(u, fn):
                fill.setdefault(u, []).append(fn)

            if p2 == 0:
                for bsc in range(SC):
                    add_fill(bsc, (lambda b=bsc: vt_chunk(b)))
            if p2 < P2 - 1:
                for i in range(8):
                    mt, n = 2 * (p2 + 1) + i // 4, i % 4
                    add_fill(16 + 4 * i, (lambda m=mt, nn=n: qk_piece(m, nn)))
            # this pair's first three t-windows: r -> normalize -> gn2 stats,
            # a few units after each window's eviction
            for tw in range(TW - 1):
                u0 = 16 * (tw + 1)
                add_fill(u0 + 3, (lambda t=tw: r_slice(p2, t)))
                add_fill(u0 + 5, (lambda t=tw: norm_tw(p2, t)))
                add_fill(u0 + 7, (lambda t=tw: gn2_stats(p2, t)))
            if p2 > 0:
                q2 = p2 - 1
                add_fill(2, (lambda q=q2: r_slice(q, TW - 1)))
                add_fill(4, (lambda q=q2: norm_tw(q, TW - 1)))
                add_fill(6, (lambda q=q2: gn2_stats(q, TW - 1)))
                add_fill(8, (lambda q=q2: gn2_combine(q)))
                for sub in range(4):
                    add_fill(10 + 2 * sub, (lambda q=q2, s=sub: gn2_apply(q, s)))
            if p2 == P2 - 1:
                for i, (mt, tw) in enumerate(
                        (m, t) for m in range(KC) for t in range(TW)):
                    add_fill(2 * i, (lambda m=mt, t=tw: xres_fetch(m, t)))
            if ht[p2] is None:
                ht[p2] = xhp.tile([128, T], BF16, tag="xh", name=f"ht{p2}")

            av_tiles = None
            for u in range(TW * SC):
                tw, bsc = u // SC, u % SC
                if bsc == 0:
                    av_tiles = (
                        avps.tile([65, 512], F32, tag="av_e", name="av_e"),
                        avps.tile([65, 512], F32, tag="av_o", name="av_o"))
                # scores for both heads (concurrent row tiles)
                sc_t = scps.tile([128, 1024], F32, tag="sc", name="scpair")
                ts = slice(512 * tw, 512 * (tw + 1))
                ss = slice(128 * bsc, 128 * (bsc + 1))
                nc.tensor.matmul(sc_t[:, 0:512], lhsT=kt[0:64, ss],
                                 rhs=qt[0:64, ts], start=True, stop=True)
                nc.tensor.matmul(sc_t[:, 512:1024], lhsT=kt[64:128, ss],
                                 rhs=qt[64:128, ts], start=True, stop=True)
                # exp on alternating engines
                pt = ptp.tile([128, 1024], BF16, tag="pt", name="pt")
                gu = 64 * p2 + u
                if (gu % 8) in DVE_SET:
                    nc.vector.tensor_scalar(
                        out=pt.bitcast(I16), in0=sc_t, scalar1=EXP_K1,
                        scalar2=EXP_K2, op0=ALU.mult, op1=ALU.add)
                else:
                    nc.scalar.activation(out=pt, in_=sc_t, func=AF.Exp,
                                         scale=SCALE2)
                # fillers (PE work to hide the exp latency)
                for fn in fill.pop(u, ()):
                    fn()
                # previous unit's AV
                if pend_av is not None:
                    flush_av(pend_av)
                pend_av = (p2, tw, bsc, pt, av_tiles)
            # leftover fillers
            for u in sorted(fill):
                for fn in fill[u]:
                    fn()

        # flush last AV + evict
        flush_av(pend_av)

        # ---- tail: last pair's last window, then proj ----
        q2 = P2 - 1
        r_slice(q2, TW - 1)
        norm_tw(q2, TW - 1)
        gn2_stats(q2, TW - 1)
        gn2_combine(q2)
        for sub in range(4):
            gn2_apply(q2, sub)

        for mt in range(KC):
            for tw in range(TW):
                ps = scps.tile([128, 1024], F32, tag="sc", name="prps")
                for k in range(KC):
                    nc.tensor.matmul(
                        ps[:, 0:512], lhsT=wp_sb[k][:, 128 * mt:128 * (mt + 1)],
                        rhs=hn[k][:, 512 * tw:512 * (tw + 1)],
                        start=(k == 0), stop=(k == KC - 1))
                ot = outp.tile([128, 512], F32, tag="ot")
                nc.vector.scalar_tensor_tensor(
                    out=ot, in0=ps[:, 0:512], scalar=bp_sb[:, mt:mt + 1],
                    in1=xres[(mt, tw)], op0=ALU.add, op1=ALU.add)
                nc.sync.dma_start(
                    out_d[128 * mt:128 * (mt + 1), 512 * tw:512 * (tw + 1)], ot)


def _build_module(reps=1):
    nc = bacc.Bacc("TRN2", target_bir_lowering=False, debug=False,
                   num_devices=NCORES)
    d = {}

    def inp(name, shape, dt=F32):
        d[name] = nc.dram_tensor(name, shape, dt, kind="ExternalInput").ap()

    inp("x", [C, T])
    inp("wqk", [C, 1024], BF16)
    inp("bqk", [1024])
    inp("wv", [C, 520], BF16)
    inp("bv", [520])
    inp("wp", [C, C], BF16)
    inp("bp", [C])
    inp("g1", [C]); inp("b1", [C]); inp("g2", [C]); inp("b2", [C])
    inp("selg", [128, 8])
    inp("selgt", [8, 128])
    inp("sel2", [8, 128], BF16)
    out_d = nc.dram_tensor("out", [C, T], F32, kind="ExternalOutput").ap()

    with tile.TileContext(nc) as tc:
        _kernel_body(nc, tc, d, out_d, reps=reps)
    nc.compile()
    _dedupe_act_loads(nc)
    return nc


def _prep_weights(w_qkv, b_qkv, w_proj, b_proj):
    w_qkv = np.asarray(w_qkv, np.float32)
    b_qkv = np.asarray(b_qkv, np.float32)
    q = [w_qkv[192 * h:192 * h + 64] for h in range(H)]
    k = [w_qkv[192 * h + 64:192 * h + 128] for h in range(H)]
    v = [w_qkv[192 * h + 128:192 * h + 192] for h in range(H)]
    qb = [b_qkv[192 * h:192 * h + 64] for h in range(H)]
    kb = [b_qkv[192 * h + 64:192 * h + 128] for h in range(H)]
    vb = [b_qkv[192 * h + 128:192 * h + 192] for h in range(H)]

    wqk_rows = []
    bqk = []
    for p in range(H // 2):
        wqk_rows += [q[2 * p], q[2 * p + 1], k[2 * p], k[2 * p + 1]]
        bqk += [qb[2 * p], qb[2 * p + 1], kb[2 * p], kb[2 * p + 1]]
    wqk = np.ascontiguousarray(np.concatenate(wqk_rows, 0).T)      # [512,1024]
    bqk = np.concatenate(bqk, 0)                                   # [1024]

    wv = np.zeros((C, 520), np.float32)
    bv = np.zeros((520,), np.float32)
    for h in range(H):
        wv[:, 65 * h:65 * h + 64] = v[h].T
        bv[65 * h:65 * h + 64] = vb[h]
        bv[65 * h + 64] = 1.0

    wp = np.ascontiguousarray(np.asarray(w_proj, np.float32).T)
    bp = np.asarray(b_proj, np.float32)

    selg = np.zeros((128, 8), np.float32)
    selg[np.arange(128), np.arange(128) // GS] = 1.0 / GS
    selgt = np.zeros((8, 128), np.float32)
    selgt[np.arange(128) // GS, np.arange(128)] = 1.0
    sel2 = np.zeros((8, 128), np.float32)
    for p in range(4):
        sel2[2 * p, 0:64] = 1.0
        sel2[2 * p + 1, 64:128] = 1.0

    bf = ml_dtypes.bfloat16
    return dict(wqk=wqk.astype(bf), bqk=bqk, wv=wv.astype(bf), bv=bv,
                wp=wp.astype(bf), bp=bp,
                selg=selg, selgt=selgt, sel2=sel2.astype(bf))


def _make_in_maps(x, gn1_gamma, gn1_beta, w_qkv, b_qkv, gn2_gamma, gn2_beta,
                  w_proj, b_proj):
    x = np.asarray(x, np.float32)
    shared = _prep_weights(w_qkv, b_qkv, w_proj, b_proj)
    shared.update(g1=np.asarray(gn1_gamma, np.float32),
                  b1=np.asarray(gn1_beta, np.float32),
                  g2=np.asarray(gn2_gamma, np.float32),
                  b2=np.asarray(gn2_beta, np.float32))
    return [dict(shared, x=np.ascontiguousarray(x[c])) for c in range(NCORES)]


def kernel(x, gn1_gamma, gn1_beta, w_qkv, b_qkv, gn2_gamma, gn2_beta, w_proj,
           b_proj):
    if "nc" not in _CACHE:
        _CACHE["nc"] = _build_module(reps=1)
    nc = _CACHE["nc"]
    in_maps = _make_in_maps(x, gn1_gamma, gn1_beta, w_qkv, b_qkv, gn2_gamma,
                            gn2_beta, w_proj, b_proj)
    res = run_bass_kernel_spmd(nc, in_maps, core_ids=list(range(NCORES)))
    out = np.stack([res.results[c]["out"] for c in range(NCORES)], 0)
    return out.astype(np.float32)


def _make_runner(nc, in_maps):
    """Cached jitted executor with device-resident inputs; per-call cost is
    dispatch + device execution only (no host transfers, no retrace)."""
    import jax
    import jax.numpy as jnp
    from jax.experimental.shard_map import shard_map
    from jax.sharding import Mesh, PartitionSpec, NamedSharding
    from concourse import bass2jax, mybir as mb

    bass2jax.install_neuronx_cc_hook()
    part_name = nc.partition_id_tensor.name if nc.partition_id_tensor else None
    in_names, out_names, out_avals, zero_outs = [], [], [], []
    for alloc in nc.m.functions[0].allocations:
        if not isinstance(alloc, mb.MemoryLocationSet):
            continue
        name = alloc.memorylocations[0].name
        if alloc.kind == "ExternalInput":
            if name != part_name:
                in_names.append(name)
        elif alloc.kind == "ExternalOutput":
            out_names.append(name)
            shape = tuple(alloc.tensor_shape)
            dtype = mb.dt.np(alloc.dtype)
            out_avals.append(jax.core.ShapedArray(shape, dtype))
            zero_outs.append(np.zeros(shape, dtype))
    n_params = len(in_names)
    all_names = in_names + out_names + ([part_name] if part_name else [])

    def _body(*args):
        operands = list(args)
        if part_name:
            operands.append(bass2jax.partition_id_tensor())
        outs = bass2jax._bass_exec_p.bind(
            *operands, out_avals=tuple(out_avals), in_names=tuple(all_names),
            out_names=tuple(out_names), lowering_input_output_aliases=(),
            sim_require_finite=True, sim_require_nnan=True, nc=nc)
        return tuple(outs)

    devices = jax.devices()[:NCORES]
    mesh = Mesh(np.asarray(devices), ("core",))
    spec = PartitionSpec("core")
    fn = jax.jit(shard_map(_body, mesh=mesh,
                           in_specs=(spec,) * (n_params + len(out_names)),
                           out_specs=(spec,) * len(out_names),
                           check_rep=False), keep_unused=True)
    sh = NamedSharding(mesh, spec)
    dev_args = [
        jax.device_put(
            np.concatenate([np.asarray(in_maps[c][nm])[None] for c in
                            range(NCORES)], 0).reshape(
                NCORES * np.asarray(in_maps[0][nm]).shape[0],
                *np.asarray(in_maps[0][nm]).shape[1:]), sh)
        for nm in in_names
    ] + [
        jax.device_put(np.zeros((NCORES * z.shape[0], *z.shape[1:]), z.dtype),
                       sh) for z in zero_outs
    ]

    def call():
        outs = fn(*dev_args)
        jax.block_until_ready(outs)
        return outs

    return call


def bench(inputs, rep_list=(1, 33), n_calls=5):
    """Estimate on-device kernel time by the slope method: per-call wall time
    of an R-rep hardware loop for two R values; the difference cancels
    dispatch overhead."""
    import time
    in_maps = _make_in_maps(**inputs)
    walls = {}
    for reps in rep_list:
        key = f"nc{reps}"
        if key not in _CACHE:
            _CACHE[key] = _build_module(reps=reps)
        runner = _make_runner(_CACHE[key], in_maps)
        runner()  # warmup (compile+load)
        times = []
        for _ in range(n_calls):
            t0 = time.time()
            runner()
            times.append(time.time() - t0)
        walls[reps] = min(times)
        print(f"reps={reps}: call walls {[f'{t*1e3:.1f}ms' for t in times]}",
              flush=True)
    lo, hi = min(rep_list), max(rep_list)
    est = (walls[hi] - walls[lo]) / (hi - lo)
    print(f"estimated per-iteration kernel time: {est * 1e9:.0f} ns")
    return est
